# revision 1
# baseline (speedup 1.0000x reference)
"""Multi-head attention (B=2, S=4096, E=1024, H=16, D=64) on 8 trn2 cores.

Sharding: data-parallel over batch x tensor-parallel over heads.
Core c handles batch b = c // 4 and head-group hg = c % 4 (4 heads each).
Each core computes QKV projections for its head-group, full attention for
its 4 heads, and the partial out-projection ctx_hg @ Wo[:, hg]^T.  The
host sums the 4 partial products per batch and adds bo + bv @ Wo^T (the
bv bias is folded out of the device kernel).

Device numerics: x and QKV weights in bf16, score/out-proj matmuls in
fp32r (TF32), P@V in bf16, accumulation always fp32.
"""

import sys

sys.path.insert(0, "/opt/trn_rl_repo")

import numpy as np
import ml_dtypes

NUM_HEADS = 16
HEAD_DIM = 64
EMBED = NUM_HEADS * HEAD_DIM
BATCH = 2
SEQ = 4096
NGROUP = 4
C = EMBED // NGROUP

_CACHE = {}

BF16 = ml_dtypes.bfloat16


def _tf32_round(x):
    x = np.ascontiguousarray(x, dtype=np.float32)
    u = x.view(np.uint32)
    r = ((u.astype(np.uint64) + 0xFFF + ((u >> 13) & 1)) & 0xFFFFE000).astype(
        np.uint32)
    return r.view(np.float32)


def _bf16(x):
    return np.ascontiguousarray(np.asarray(x, dtype=np.float32).astype(BF16))


def _build_nc():
    if "nc" not in _CACHE:
        import mha_bass_v2
        _CACHE["nc"] = mha_bass_v2.build(S=SEQ, E=EMBED, C=C)
    return _CACHE["nc"]


def make_in_maps(query, key, value, Wq, bq, Wk, bk, Wv, bv, Wo, bo):
    query = np.asarray(query, dtype=np.float32)
    key = np.asarray(key, dtype=np.float32)
    value = np.asarray(value, dtype=np.float32)
    Wq, Wk, Wv, Wo = (np.asarray(w, dtype=np.float32) for w in (Wq, Wk, Wv, Wo))
    bq, bk, bv = (np.asarray(b, dtype=np.float32) for b in (bq, bk, bv))

    xT = {("q", b): _bf16(query[b].T) for b in range(BATCH)}
    xT.update({("k", b): _bf16(key[b].T) for b in range(BATCH)})
    xT.update({("v", b): _bf16(value[b].T) for b in range(BATCH)})

    wslc = {}
    for g in range(NGROUP):
        sl = slice(g * C, (g + 1) * C)
        wslc[("q", g)] = _bf16(Wq[sl, :].T)
        wslc[("k", g)] = _bf16(Wk[sl, :].T)
        wslc[("v", g)] = _bf16(Wv[sl, :].T)
        wslc[("o", g)] = _tf32_round(Wo[:, sl].T)

    in_maps = []
    for core in range(8):
        b, g = core // NGROUP, core % NGROUP
        sl = slice(g * C, (g + 1) * C)
        in_maps.append({
            "xTq": xT[("q", b)], "xTk": xT[("k", b)], "xTv": xT[("v", b)],
            "wqT": wslc[("q", g)], "wkT": wslc[("k", g)],
            "wvT": wslc[("v", g)], "woT": wslc[("o", g)],
            "bq": np.ascontiguousarray(bq[sl].reshape(C, 1)),
            "bk": np.ascontiguousarray(bk[sl].reshape(C, 1)),
            "bv": np.ascontiguousarray(bv[sl].reshape(C, 1)),
        })
    return in_maps


def kernel(query, key, value, Wq, bq, Wk, bk, Wv, bv, Wo, bo):
    from concourse.bass_utils import run_bass_kernel_spmd

    in_maps = make_in_maps(query, key, value, Wq, bq, Wk, bk, Wv, bv, Wo, bo)
    nc = _build_nc()
    res = run_bass_kernel_spmd(nc, in_maps, core_ids=list(range(8)))

    bo = np.asarray(bo, dtype=np.float32)
    bv = np.asarray(bv, dtype=np.float32)
    Wo = np.asarray(Wo, dtype=np.float32)
    out = np.zeros((BATCH, SEQ, EMBED), dtype=np.float32)
    for core in range(8):
        out[core // NGROUP] += res.results[core]["out"]
    # device omits the bv add: (ctx+bv)@Wo^T = ctx@Wo^T + bv@Wo^T
    out += bo + bv @ Wo.T
    return out


# ---------------------------------------------------------------------------
# The Bass kernel builder is embedded below and written to a module file so
# kernel.py stays fully self-contained.
# ---------------------------------------------------------------------------
_MHA_BASS_SRC = '"""Bass/Tile MHA kernel v2.2 for one core: one (batch, head-group) slice.\n\nThe Activation engine (exp) is the hard floor (512 exps x ~1.1us =\n570us); everything else is arranged to keep it 100% busy:\n  * pair-split attention (qt -> pair -> kvb) with the score PSUM\n    double-buffered (2 slots x 2 banks) so scores(k+1) never wait on\n    exp(k);\n  * software-pipelined pair boundaries: the next pair\'s first scores\n    are emitted before the previous pair\'s PV-trail flush, so the exp\n    stream never drains at a boundary;\n  * ctx accumulators drained to SBUF right after each pair so their\n    PSUM slots (2 x 1 bank) recycle without waiting on the normalize\n    chain; normalize runs per-pair;\n  * dedicated 1-bank PSUM slots for Q-proj and out-proj; the tail\n    out-proj (after the last exp) borrows the then-idle score slots so\n    it pipelines instead of ping-ponging on one bank;\n  * K/V projections in a DMA-overlapped pre-phase, Q projected one\n    chunk (=one q-tile) ahead, spread as per-ke matmul hooks inside the\n    attention stream; out-proj of qt interleaved as 8 half-units into\n    qt+1\'s pair0 stream, ordered so the single ops slot never stalls;\n  * two busy hardware DMA queues: Sync streams the inputs, GpSimd\n    carries wo, the output tiles and the small normalize round-trips.\n\nLayout contract (host-prepared):\n  xTq, xTk, xTv : [E, S]   bf16  (input slice, transposed on host)\n  wqT, wkT, wvT : [E, C]   bf16  (weight head-group slice, transposed)\n  woT           : [C, E]   f32r  (out-proj slice, transposed, tf32-rounded)\n  bq, bk, bv    : [C, 1]   f32\n  out           : [S, E]   f32   (partial out-proj product, pre-bo)\n"""\n\nimport contextlib\n\nimport concourse.bass as bass\nimport concourse.mybir as mybir\nimport concourse.tile as tile\nfrom concourse import bacc\n\nF32 = mybir.dt.float32\nF32R = mybir.dt.float32r\nBF16 = mybir.dt.bfloat16\nAF = mybir.ActivationFunctionType\n\n\ndef build(S=4096, E=1024, C=256):\n    D = 64\n    NH = C // D            # 4 heads per core\n    NPAIR = NH // 2        # 2 pairs\n    KE = E // 128          # contraction tiles for projections\n    QT = 512               # q-tile\n    NQT = S // QT\n    CHUNK = 512            # s-chunk for projections (= QT so Q chunk == q-tile)\n    NCH = S // CHUNK\n    NKVB = S // 128        # kv blocks\n    TRAIL = 2              # PV trails exp by this many kv blocks\n    SCALE = 1.0 / 8.0      # 1/sqrt(D)\n\n    nc = bacc.Bacc("TRN2", target_bir_lowering=False)\n\n    xT = {n: nc.dram_tensor(f"xT{n}", [E, S], BF16, kind="ExternalInput")\n          for n in ("q", "k", "v")}\n    wT = {n: nc.dram_tensor(f"w{n}T", [E, C], BF16, kind="ExternalInput")\n          for n in ("q", "k", "v")}\n    woT = nc.dram_tensor("woT", [C, E], F32R, kind="ExternalInput")\n    b_in = {n: nc.dram_tensor(f"b{n}", [C, 1], F32, kind="ExternalInput")\n            for n in ("q", "k", "v")}\n    out = nc.dram_tensor("out", [S, E], F32, kind="ExternalOutput")\n\n    with tile.TileContext(nc) as tc, contextlib.ExitStack() as ctx:\n        p_w = ctx.enter_context(tc.tile_pool(name="w", bufs=1))\n        p_xt = ctx.enter_context(tc.tile_pool(name="xt", bufs=4))\n        p_qk = ctx.enter_context(tc.tile_pool(name="qk", bufs=1))\n        p_v = ctx.enter_context(tc.tile_pool(name="v", bufs=1))\n        p_pt = ctx.enter_context(tc.tile_pool(name="pt", bufs=8))\n        p_cts = ctx.enter_context(tc.tile_pool(name="cts", bufs=4))\n        p_late = ctx.enter_context(tc.tile_pool(name="late", bufs=2))\n        p_ost = ctx.enter_context(tc.tile_pool(name="ost", bufs=3))\n        p_nrm = ctx.enter_context(tc.tile_pool(name="nrm", bufs=1))\n        # PSUM: one 3-slot ring of 2-bank tiles (scores + Q-proj + out-proj\n        # all share it) + 2x1-bank ctx accumulators = 8 banks exactly.\n        # Three score slots give TensorE two kv-blocks of lookahead, so\n        # inserted out-proj/Q-proj matmuls no longer delay the exp stream.\n        ps_big = ctx.enter_context(tc.tile_pool(name="psb", bufs=3, space="PSUM"))\n        ps_ctx = ctx.enter_context(tc.tile_pool(name="psc", bufs=2, space="PSUM"))\n\n        # ---- weights + biases to SBUF ----\n        w_sb = {}\n\n        def load_w(n, eng=None):\n            t = p_w.tile([128, KE, C], BF16, tag=f"w{n}", name=f"w{n}")\n            (eng or nc.sync).dma_start(\n                out=t, in_=wT[n].rearrange("(ke p) c -> p ke c", p=128))\n            w_sb[n] = t\n\n        def load_b(n):\n            t = p_w.tile([128, C // 128], F32, tag=f"b{n}", name=f"b{n}")\n            nc.sync.dma_start(\n                out=t, in_=b_in[n].rearrange("(m p) one -> p (m one)", p=128))\n            return t\n\n        # wk leads the scalar weight queue (same proven pattern as wv/wo)\n        # so it lands in parallel with xk0 on the sync queue\n        load_w("k", nc.scalar)\n        bias_sb = {"k": load_b("k")}\n        # wo + wv ride the scalar queue (Act engine idle until the first\n        # exp), xv chunks the gpsimd queue, xk/xq the sync queue -- three\n        # parallel DMA channels for the pre-phase.\n        w_sb["v"] = p_w.tile([128, KE, C], BF16, tag="wv", name="wv")\n        nc.scalar.dma_start(\n            out=w_sb["v"], in_=wT["v"].rearrange("(ke p) c -> p ke c", p=128))\n        wo_sb = p_w.tile([128, C // 128, E], F32R, tag="wo")\n        nc.scalar.dma_start(\n            out=wo_sb, in_=woT.rearrange("(ct p) e -> p ct e", p=128))\n\n        # ---- persistent activation tensors ----\n        Kt = p_qk.tile([128, NPAIR, S], F32R, tag="Kt")\n        Qt = p_qk.tile([128, NPAIR, S], F32R, tag="Qt")\n        Vp = [p_v.tile([128, NKVB, D + 1], BF16, tag=f"Vp{h}", name=f"Vp{h}")\n              for h in range(NH)]\n        for h in range(NH):\n            nc.vector.memset(Vp[h][:, :, D:D + 1], 1.0)\n\n        # ---- projection helpers ----\n        def load_x_chunk(n, ch):\n            t = p_xt.tile([128, KE, CHUNK], BF16, tag="xt", name=f"xt_{n}{ch}")\n            nc.sync.dma_start(\n                out=t,\n                in_=xT[n].rearrange("(ke p) s -> p ke s", p=128)\n                [:, :, ch * CHUNK:(ch + 1) * CHUNK])\n            return t\n\n        def proj_qk_chunk(n, dst, ch, pool, xt_t=None):\n            if xt_t is None:\n                xt_t = load_x_chunk(n, ch)\n            for mt in range(NPAIR):\n                ps = pool.tile([128, CHUNK], F32, tag=pool.name,\n                               name=f"ps_{n}{ch}_{mt}")\n                for ke in range(KE):\n                    nc.tensor.matmul(\n                        ps,\n                        lhsT=w_sb[n][:, ke, mt * 128:(mt + 1) * 128],\n                        rhs=xt_t[:, ke, :],\n                        start=(ke == 0), stop=(ke == KE - 1))\n                nc.vector.tensor_scalar_add(\n                    out=dst[:, mt, ch * CHUNK:(ch + 1) * CHUNK],\n                    in0=ps, scalar1=bias_sb[n][:, mt:mt + 1])\n\n        def proj_v_chunk(ch, xt_t):\n            for st in range(CHUNK // 128):\n                ps = ps_big.tile([128, C], F32, tag="psb",\n                                 name=f"ps_v{ch}_{st}")\n                for ke in range(KE):\n                    nc.tensor.matmul(\n                        ps,\n                        lhsT=xt_t[:, ke, st * 128:(st + 1) * 128],\n                        rhs=w_sb["v"][:, ke, :],\n                        start=(ke == 0), stop=(ke == KE - 1))\n                kvb = ch * (CHUNK // 128) + st\n                for h in range(NH):\n                    nc.vector.tensor_copy(\n                        out=Vp[h][:, kvb, 0:D],\n                        in_=ps[:, h * D:(h + 1) * D])\n\n        # ---- pre-phase: K and V fully projected (DMA-overlapped) ----\n        xk0 = load_x_chunk("k", 0)\n        xv0 = load_x_chunk("v", 0)\n        proj_qk_chunk("k", Kt, 0, ps_ctx, xk0)\n        proj_v_chunk(0, xv0)\n        for ch in range(1, NCH):\n            proj_qk_chunk("k", Kt, ch, ps_ctx, load_x_chunk("k", ch))\n            proj_v_chunk(ch, load_x_chunk("v", ch))\n        load_w("q")\n        bias_sb["q"] = load_b("q")\n        proj_qk_chunk("q", Qt, 0, ps_big, load_x_chunk("q", 0))\n\n        # ---- out-projection half-unit: one 128-row q block x 512 cols.\n        # Emitted in two single-matmul stages so each TensorE insertion\n        # into the attention stream stays under the per-kvb slack.\n        def outproj_mm(qt, sub, nt, ct, ctxT, ops):\n            nc.tensor.matmul(\n                ops,\n                lhsT=ctxT[:, ct, sub * 128:(sub + 1) * 128],\n                rhs=wo_sb[:, ct, nt * QT:(nt + 1) * QT],\n                start=(ct == 0), stop=(ct == C // 128 - 1))\n\n        def outproj_fin(qt, sub, nt, ost, ops, eng=None):\n            nc.vector.tensor_copy(ost[:, nt * QT:(nt + 1) * QT], ops)\n            if nt == E // QT - 1:\n                (eng or nc.gpsimd).dma_start(\n                    out=out[qt * QT + sub * 128: qt * QT + (sub + 1) * 128, :],\n                    in_=ost)\n\n        def outproj_half(qt, sub, nt, ctxT, ost, pool, eng=None):\n            ops = pool.tile([128, QT], F32, tag=pool.name,\n                            name=f"op{qt}_{sub}_{nt}")\n            for ct in range(C // 128):\n                outproj_mm(qt, sub, nt, ct, ctxT, ops)\n            outproj_fin(qt, sub, nt, ost, ops, eng)\n\n        def new_ost(qt, sub):\n            return p_ost.tile([128, E], F32, tag="ost", name=f"ost{qt}_{sub}")\n\n        # ---- per-pair normalize, two stages.  Stage 1 (at the flush):\n        # denominator extraction, reciprocal, broadcast.  Stage 2 (a hook\n        # ~10 kv-blocks later, when bca is ready): the two multiplies --\n        # deferred so they never head-of-line-block the DVE queue.  The\n        # bv bias is folded into the host-side output sum (ctx+bv)@Wo^T =\n        # ctx@Wo^T + bv@Wo^T, so no bias add here.  gpsimd runs only\n        # partition_broadcast + DMA triggers (tensor ops there would\n        # force a ~7us microcode library swap per call).\n        def normalize_stage1(qt, pr, cs):\n            dsp = p_nrm.tile([64, NH * QT // 128], F32, tag=f"dsp{pr}",\n                             name=f"dsp{qt}_{pr}")\n            for i in range(2):\n                nc.gpsimd.dma_start(\n                    out=dsp[32 * i:32 * (i + 1), :], in_=cs[i][D:D + 1, :])\n            nc.vector.reciprocal(dsp, dsp)\n            r0 = p_nrm.tile([1, 2, QT], F32, tag=f"r0{pr}",\n                            name=f"r0_{qt}_{pr}")\n            nc.gpsimd.dma_start(out=r0, in_=dsp)\n            bca = p_nrm.tile([D, 2, QT], F32, tag=f"bca{pr}",\n                             name=f"bca{qt}_{pr}")\n            nc.gpsimd.partition_broadcast(bca, r0[0:1, :, :])\n            return bca\n\n        def normalize_stage2(st2):\n            qt, pr, cs, bca, ctxT = st2\n            nc.vector.tensor_mul(\n                ctxT[0:D, pr, :], cs[0][0:D, :], bca[:, 0, :])\n            tmp = p_nrm.tile([D, QT], F32R, tag=f"nrm{pr}", bufs=2,\n                             name=f"nrm{qt}_{pr}")\n            nc.vector.tensor_mul(tmp, cs[1][0:D, :], bca[:, 1, :])\n            nc.gpsimd.dma_start(out=ctxT[D:2 * D, pr, :], in_=tmp)\n\n        # ---- attention ----\n        # carry: previous pair\'s unfinished work, flushed TRAIL kv-blocks\n        # into the next pair so the exp stream never drains.\n        carry = {}\n\n        pending_st2 = []  # deferred normalize stage-2 closures\n\n        def flush_carry_partial():\n            if carry and len(carry["pend"]) > 1:\n                carry["emit_pv"](*carry["pend"].pop(0))\n\n        def flush_carry():\n            if not carry:\n                return\n            for it in carry["pend"]:\n                carry["emit_pv"](*it)\n            # denominator rows drain first so the dn DMAs overlap the\n            # (much larger) ctx-row drains\n            cs = []\n            for i in range(2):\n                t = p_cts.tile([D + 1, QT], F32, tag="cts",\n                               name=f"cs{carry[\'qt\']}_{2 * carry[\'pr\'] + i}")\n                nc.vector.tensor_copy(\n                    t[D:D + 1, :], carry["ctx_h"][i][D:D + 1, :])\n                cs.append(t)\n            for i in range(2):\n                nc.vector.tensor_copy(\n                    cs[i][0:D, :], carry["ctx_h"][i][0:D, :])\n            bca = normalize_stage1(carry["qt"], carry["pr"], cs)\n            pending_st2.append(\n                (carry["qt"], carry["pr"], cs, bca, carry["ctxT"]))\n            carry.clear()\n\n        # hook order: same ops-slot users 4 kv-blocks apart\n        HOOK_KVB = [16, 18, 20, 22, 24, 26, 28, 30]\n        HOOK_UNIT = [(0, 0), (1, 0), (0, 1), (1, 1),\n                     (2, 0), (3, 0), (2, 1), (3, 1)]\n\n        prev = None  # qt-1\'s ctxT\n\n        for qt in range(NQT):\n            ctxT = p_late.tile([128, C // 128, QT], F32R, tag="ctxT",\n                               name=f"ctxT{qt}")\n            for pr in range(NPAIR):\n                ctx_h = [ps_ctx.tile([D + 1, QT], F32, tag="psc",\n                                     name=f"ctx{qt}_{pr}_{i}")\n                         for i in range(2)]\n                pend = []\n                # prefetch next q-tile\'s x chunk; projected at pair end\n                if pr == 0 and qt + 1 < NQT:\n                    xq_next = load_x_chunk("q", qt + 1)\n\n                def emit_pv(kvb, pt, ctx_h=ctx_h, pr=pr):\n                    for i in range(2):\n                        nc.tensor.matmul(\n                            ctx_h[i],\n                            lhsT=Vp[2 * pr + i][:, kvb, :],\n                            rhs=pt[:, i * QT:(i + 1) * QT],\n                            start=(kvb == 0), stop=(kvb == NKVB - 1))\n\n                hooks = dict(zip(HOOK_KVB, HOOK_UNIT)) if (\n                    pr == 0 and prev is not None) else {}\n                osts = {}\n\n                for kvb in range(NKVB):\n                    bigt = ps_big.tile([128, 2 * QT], F32, tag="psb",\n                                       name=f"big{qt}_{pr}_{kvb}")\n                    for i in range(2):\n                        nc.tensor.matmul(\n                            bigt[:, i * QT:(i + 1) * QT],\n                            lhsT=Kt[i * D:(i + 1) * D, pr,\n                                    kvb * 128:(kvb + 1) * 128],\n                            rhs=Qt[i * D:(i + 1) * D, pr,\n                                   qt * QT:(qt + 1) * QT],\n                            start=True, stop=True,\n                            tile_position=(i * D, 0))\n                    pt = p_pt.tile([128, 2 * QT], BF16, tag="pt",\n                                   name=f"pt{qt}_{pr}_{kvb}")\n                    nc.scalar.activation(\n                        out=pt, in_=bigt, func=AF.Exp, scale=SCALE)\n                    pend.append((kvb, pt))\n                    if kvb == TRAIL - 1:\n                        flush_carry_partial()\n                    elif kvb == TRAIL:\n                        flush_carry()\n                    elif kvb == 10 and pending_st2:\n                        normalize_stage2(pending_st2.pop(0))\n                    if len(pend) > TRAIL:\n                        emit_pv(*pend.pop(0))\n                    if kvb in hooks:\n                        sub, nt = hooks[kvb]\n                        if nt == 0:\n                            osts[sub] = new_ost(qt - 1, sub)\n                        outproj_half(qt - 1, sub, nt, prev, osts[sub], ps_big)\n                    # Q-proj of qt+1 as two half-lumps inside pair1: the\n                    # ring\'s pre-built exps cover each ~2us TensorE detour\n                    if pr == 1 and qt + 1 < NQT and kvb in (3, 18):\n                        mt = 0 if kvb == 3 else 1\n                        qps = ps_big.tile([128, CHUNK], F32, tag="psb",\n                                          name=f"ps_q{qt + 1}_{mt}")\n                        for ke in range(KE):\n                            nc.tensor.matmul(\n                                qps,\n                                lhsT=w_sb["q"][:, ke,\n                                               mt * 128:(mt + 1) * 128],\n                                rhs=xq_next[:, ke, :],\n                                start=(ke == 0), stop=(ke == KE - 1))\n                        nc.vector.tensor_scalar_add(\n                            out=Qt[:, mt,\n                                   (qt + 1) * CHUNK:(qt + 2) * CHUNK],\n                            in0=qps, scalar1=bias_sb["q"][:, mt:mt + 1])\n\n                carry.update(qt=qt, pr=pr, pend=list(pend), ctx_h=ctx_h,\n                             emit_pv=emit_pv, ctxT=ctxT)\n            prev = ctxT\n\n        # tail: flush the last pair, then its out-proj on the idle slots\n        flush_carry()\n        while pending_st2:\n            normalize_stage2(pending_st2.pop(0))\n        # alternate tail output DMAs over the gpsimd and (now idle)\n        # scalar queues so the final write-back isn\'t single-queue serial\n        for sub in range(QT // 128):\n            ost = new_ost(NQT - 1, sub)\n            eng = nc.scalar if sub % 2 else nc.gpsimd\n            for nt in range(E // QT):\n                outproj_half(NQT - 1, sub, nt, prev, ost, ps_big, eng)\n\n    nc.compile()\n    return nc\n'

import os as _os
import tempfile as _tempfile

_moddir = _tempfile.mkdtemp(prefix="mha_bass_v2_")
with open(_os.path.join(_moddir, "mha_bass_v2.py"), "w") as _f:
    _f.write(_MHA_BASS_SRC)
sys.path.insert(0, _moddir)

